# revision 1
# baseline (speedup 1.0000x reference)
"""DOSAConLoss Trainium2 kernel.

result = mean(base) * mean(1 + ALPHA * density)
       = mean(base) * (1 + ALPHA * (N/1024) / max_hist)

since sum(hist) == N exactly (every box center lands in one bin).

Per core (8-way data parallel over N): compute
  - per-partition partial sums of base  (acc_out [128, n_tiles])
  - partial 32x32 histogram of target box centers (hist_out [32, 32])
Host combines: sums acc, sums hists (minus padding), applies the scalar formula.

Math rewrite (validated vs reference in fp64/fp32):
  dx=x1-x2, W=w1+w2, dW=w1-w2 (same for y/h)
  iw4 = relu(W - max(|2dx|,|dW|)) = 2*iw ; inter4 = iw4*ih4 = 4*inter
  union = a1+a2 - inter4/4 (+eps)   ; iou = inter4 * 0.25/(union+eps)
  cw2 = W + mx = 2*cw ; c24 = cw2^2+ch2^2 = 4*c2 ; rho4 = (2dx)^2+(2dy)^2
  rho2/c2 == rho4/c24
  atan(w/h) range-reduced: q~ = min(w,h)/max(w,h) in [0,1];
     theta = atan(q~) + [w>h]*(pi/2 - 2*atan(q~))
  v = ((th2-th1)*2/pi)^2 ; a = v/(v-iou+1+eps)
  ciou = iou - rho4/c24 - v^2/(v-iou+1+eps)
  base = (1-ciou)^3 / (w2*h2 + 1e-7)
Reciprocals via exp(-ln(x)) (ACT Reciprocal is disallowed in bass).
Histogram: floor(32*x) via magic-number RNE rounding (mod/divide are not
  ISA-legal on DVE); x-side: 32 bin-major bf16 one-hot tensor_scalar
  is_equal ops; y-side packed to 16 rows with radix-512 parity weights
  (uy[m] = [floor(16y)==m] * (1 + 511*(gy mod 2))), so TensorE accumulates
  psum[16,32] += uy[:,:,t].T @ ohx[:,:,t] per 128-box column, in 4
  accumulation groups of 2 tiles (cell counts stay < 512 for exact radix
  decode). Host decodes the packed groups, exactly relocating the ~1e-6
  fraction of fp-tie boxes where the device trick-bin differs from floor.
"""

import numpy as np

import concourse.bass as bass
import concourse.bacc as bacc
import concourse.mybir as mybir
import concourse.tile as tile
from concourse import bass_utils

# The act-table-load chooser picks the first set containing each function,
# which puts Ln in `natural_log` and Exp in `exp_and_others`, forcing a
# ~2.7us table switch at every Ln->Exp pair (we use exp(-ln(x)) for all
# reciprocals). Hide Ln/Exp from the single-function sets so the chooser
# lands on `natural_log_exp_and_others` (set ids keep their act_info.json
# positions; only membership is masked).
_orig_get_act_tables = bacc.get_activation_tables


def _patched_get_act_tables(arch):
    t = {k: set(v) for k, v in _orig_get_act_tables(arch).items()}
    t.get("natural_log", set()).discard(mybir.ActivationFunctionType.Ln)
    t.get("exp_and_others", set()).discard(mybir.ActivationFunctionType.Exp)
    t.get("exp_and_friends", set()).discard(mybir.ActivationFunctionType.Exp)
    return t


bacc.get_activation_tables = _patched_get_act_tables

F32 = mybir.dt.float32
BF16 = mybir.dt.bfloat16
AF = mybir.ActivationFunctionType
OP = mybir.AluOpType

GRID = 32
ALPHA = 1.5
EPS = 1e-7
PI = float(np.pi)
MAGIC = float(2 ** 23)

N_CORES = 8
N_TOTAL = 4_000_000
NB_CORE = 524_288            # padded boxes per core: 128 * 4096
PAD_BOX = (0.5, 0.5, 1.0, 1.0)  # pred==targ box -> base contribution ~1e-21, bin (16,16)

# GPSIMD offload set for 2-input tensor_tensor ops (tune via profile)
# (POOL TensorTensor float ops: only add/subtract/mult are ISA-legal)
GPS_OPS = {"asum", "cw2", "ch2", "c24", "rho4", "th2a", "th1a", "dat", "term2", "s12"}


def build_nc(NB, T=512, Tc=512, gps=True):
    """Build the per-core Bass program. NB must equal n_tiles*128*T."""
    n_tiles = NB // (128 * T)
    assert NB == n_tiles * 128 * T
    n_chunks = T // Tc
    assert T == n_chunks * Tc

    nc = bacc.Bacc("TRN2", target_bir_lowering=False, debug=False)
    pred_d = nc.dram_tensor("pred_boxes", [NB, 4], F32, kind="ExternalInput")
    targ_d = nc.dram_tensor("target_boxes", [NB, 4], F32, kind="ExternalInput")
    acc_d = nc.dram_tensor("acc_out", [128, n_tiles], F32, kind="ExternalOutput")
    n_grp_ = max(1, (NB // (128 * T)) // 2)
    hist_d = nc.dram_tensor("hist_out", [GRID // 2, GRID * n_grp_], F32, kind="ExternalOutput")

    pred_v = pred_d.ap().rearrange("(n p t) c -> n p (t c)", p=128, t=T)
    targ_v = targ_d.ap().rearrange("(n p t) c -> n p (t c)", p=128, t=T)

    def eng(name):
        return nc.gpsimd if (gps and name in GPS_OPS) else nc.vector

    with tile.TileContext(nc) as tc:
        with (
            tc.tile_pool(name="inp", bufs=3) as inp,
            tc.tile_pool(name="tmp", bufs=2) as tmp,
            tc.tile_pool(name="ohp", bufs=2) as ohp,
            tc.tile_pool(name="cst", bufs=1) as cst,
            tc.tile_pool(name="psp", bufs=1, space="PSUM") as psp,
        ):
            bias_tiles = {}

            def bias_ap(val):
                if val not in bias_tiles:
                    t = cst.tile([128, 1], F32, name=f"bias{len(bias_tiles)}")
                    nc.vector.memset(t[:], val)
                    bias_tiles[val] = t[:]
                return bias_tiles[val]
            acc_sb = cst.tile([128, n_tiles], F32)
            n_grp = max(1, n_tiles // 2)
            hist_sb = cst.tile([GRID // 2, GRID * n_grp], F32)
            ps_g = [psp.tile([GRID // 2, GRID], F32, name=f"ps{g}") for g in range(n_grp)]

            mm_i = 0
            total_mms = NB // 128

            # Temp slot allocator: long-lived temps get dedicated tags;
            # short-lived ones rotate through NGEN generic tags (bufs=2 each,
            # Tile inserts WAR deps on slot reuse). Max temp lifetime must be
            # < 2*NGEN generic allocations.
            NGEN = 12
            DEDICATED = {"a2t", "iou", "term1"}
            gen_counter = [0]

            for n in range(n_tiles):
                pt = inp.tile([128, 4 * T], F32, tag="pred")
                tt = inp.tile([128, 4 * T], F32, tag="targ")
                nc.sync.dma_start(pt[:], pred_v[n])
                nc.sync.dma_start(tt[:], targ_v[n])
                p3 = pt.rearrange("p (t c) -> p c t", c=4)
                t3 = tt.rearrange("p (t c) -> p c t", c=4)
                x1, y1, w1, h1 = p3[:, 0], p3[:, 1], p3[:, 2], p3[:, 3]
                x2, y2, w2, h2 = t3[:, 0], t3[:, 1], t3[:, 2], t3[:, 3]

                def t_(tag):
                    if tag in DEDICATED:
                        return tmp.tile([128, T], F32, tag=tag, name=tag)[:]
                    i = gen_counter[0] % NGEN
                    gen_counter[0] += 1
                    return tmp.tile([128, T], F32, tag=f"g{i}", name=tag)[:]

                dx, dy = t_("dx"), t_("dy")
                W, dW, H, dH = t_("W"), t_("dW"), t_("H"), t_("dH")
                nc.vector.tensor_tensor(dx, x1, x2, OP.subtract)
                nc.vector.tensor_tensor(dy, y1, y2, OP.subtract)
                nc.vector.tensor_tensor(W, w1, w2, OP.add)
                nc.vector.tensor_tensor(dW, w1, w2, OP.subtract)
                nc.vector.tensor_tensor(H, h1, h2, OP.add)
                nc.vector.tensor_tensor(dH, h1, h2, OP.subtract)
                a2t, a1t, asum = t_("a2t"), t_("a1t"), t_("asum")
                nc.vector.tensor_tensor(a2t, w2, h2, OP.mult)
                nc.vector.tensor_tensor(a1t, w1, h1, OP.mult)
                eng("asum").tensor_tensor(asum, a1t, a2t, OP.add)

                adx, ady, adW, adH = t_("adx"), t_("ady"), t_("adW"), t_("adH")
                nc.scalar.activation(adx, dx, AF.Abs, scale=2.0)
                nc.scalar.activation(ady, dy, AF.Abs, scale=2.0)
                nc.scalar.activation(adW, dW, AF.Abs)
                nc.scalar.activation(adH, dH, AF.Abs)

                mx, my = t_("mx"), t_("my")
                nc.vector.tensor_tensor(mx, adx, adW, OP.max)
                nc.vector.tensor_tensor(my, ady, adH, OP.max)

                iw4, ih4, ihc, inter4 = t_("iw4"), t_("ih4"), t_("ihc"), t_("inter4")
                nc.vector.scalar_tensor_tensor(iw4, mx, -1.0, W, OP.mult, OP.add)
                nc.vector.scalar_tensor_tensor(ih4, my, -1.0, H, OP.mult, OP.add)
                nc.vector.tensor_scalar(ihc, ih4, 0.0, None, OP.max)
                nc.vector.scalar_tensor_tensor(inter4, iw4, 0.0, ihc, OP.max, OP.mult)

                u = t_("u")
                nc.vector.scalar_tensor_tensor(u, inter4, -0.25, asum, OP.mult, OP.add)
                lnu, r_u = t_("lnu"), t_("r_u")
                nc.scalar.activation(lnu, u, AF.Ln, scale=4.0, bias=bias_ap(4 * EPS))
                nc.scalar.activation(r_u, lnu, AF.Exp, scale=-1.0)
                iou = t_("iou")
                nc.vector.tensor_tensor(iou, inter4, r_u, OP.mult)

                cw2, ch2 = t_("cw2"), t_("ch2")
                eng("cw2").tensor_tensor(cw2, W, mx, OP.add)
                eng("ch2").tensor_tensor(ch2, H, my, OP.add)
                scw, sch, sdx, sdy = t_("scw"), t_("sch"), t_("sdx"), t_("sdy")
                nc.scalar.activation(scw, cw2, AF.Square)
                nc.scalar.activation(sch, ch2, AF.Square)
                nc.scalar.activation(sdx, adx, AF.Square)
                nc.scalar.activation(sdy, ady, AF.Square)
                c24, rho4 = t_("c24"), t_("rho4")
                eng("c24").tensor_tensor(c24, scw, sch, OP.add)
                eng("rho4").tensor_tensor(rho4, sdx, sdy, OP.add)
                lnc, r_c = t_("lnc"), t_("r_c")
                nc.scalar.activation(lnc, c24, AF.Ln, bias=bias_ap(4 * EPS))
                nc.scalar.activation(r_c, lnc, AF.Exp, scale=-1.0)
                term1 = t_("term1")
                nc.vector.tensor_tensor(term1, rho4, r_c, OP.mult)

                # arctan(w/h) for both boxes, range-reduced to [0,1]
                mn2, mxx2, mn1, mxx1 = t_("mn2"), t_("mxx2"), t_("mn1"), t_("mxx1")
                nc.vector.tensor_tensor(mn2, w2, h2, OP.min)
                nc.vector.tensor_tensor(mxx2, w2, h2, OP.max)
                nc.vector.tensor_tensor(mn1, w1, h1, OP.min)
                nc.vector.tensor_tensor(mxx1, w1, h1, OP.max)
                lm2, rr2, lm1, rr1 = t_("lm2"), t_("rr2"), t_("lm1"), t_("rr1")
                nc.scalar.activation(lm2, mxx2, AF.Ln, bias=bias_ap(1e-30))
                nc.scalar.activation(rr2, lm2, AF.Exp, scale=-1.0)
                nc.scalar.activation(lm1, mxx1, AF.Ln, bias=bias_ap(1e-30))
                nc.scalar.activation(rr1, lm1, AF.Exp, scale=-1.0)
                qt2, qt1, sel2, sel1 = t_("qt2"), t_("qt1"), t_("sel2"), t_("sel1")
                nc.vector.tensor_tensor(qt2, mn2, rr2, OP.mult)
                nc.vector.tensor_tensor(qt1, mn1, rr1, OP.mult)
                nc.vector.tensor_tensor(sel2, w2, h2, OP.is_gt)
                nc.vector.tensor_tensor(sel1, w1, h1, OP.is_gt)
                at2, at1 = t_("at2"), t_("at1")
                nc.scalar.activation(at2, qt2, AF.Arctan)
                nc.scalar.activation(at1, qt1, AF.Arctan)
                # theta_i = |sel_i*pi/2 - at_i|  (== atan(w_i/h_i))
                a2d, a1d, th2, th1 = t_("a2d"), t_("a1d"), t_("th2"), t_("th1")
                nc.vector.scalar_tensor_tensor(a2d, sel2, PI / 2, at2, OP.mult, OP.subtract)
                nc.vector.scalar_tensor_tensor(a1d, sel1, PI / 2, at1, OP.mult, OP.subtract)
                nc.scalar.activation(th2, a2d, AF.Abs)
                nc.scalar.activation(th1, a1d, AF.Abs)
                dat = t_("dat")
                eng("dat").tensor_tensor(dat, th2, th1, OP.subtract)
                vv = t_("vv")
                nc.scalar.activation(vv, dat, AF.Square, scale=2.0 / PI)

                den0 = t_("den0")
                nc.vector.tensor_tensor(den0, vv, iou, OP.subtract)
                lnden, rden, v2 = t_("lnden"), t_("rden"), t_("v2")
                nc.scalar.activation(lnden, den0, AF.Ln, bias=bias_ap(1.0 + EPS))
                nc.scalar.activation(rden, lnden, AF.Exp, scale=-1.0)
                nc.scalar.activation(v2, vv, AF.Square)
                term2, s12, z = t_("term2"), t_("s12"), t_("z")
                eng("term2").tensor_tensor(term2, v2, rden, OP.mult)
                eng("s12").tensor_tensor(s12, term1, term2, OP.add)
                nc.vector.scalar_tensor_tensor(z, iou, -1.0, s12, OP.mult, OP.add)

                om2, lnsw, sw = t_("om2"), t_("lnsw"), t_("sw")
                nc.scalar.activation(om2, z, AF.Square, bias=bias_ap(1.0))
                nc.scalar.activation(lnsw, a2t, AF.Ln, bias=bias_ap(1e-7))
                nc.scalar.activation(sw, lnsw, AF.Exp, scale=-1.0)
                om3, baset = t_("om3"), t_("baset")
                nc.vector.scalar_tensor_tensor(om3, z, 1.0, om2, OP.add, OP.mult)
                nc.vector.scalar_tensor_tensor(
                    baset, om3, 0.0, sw, OP.add, OP.mult,
                    accum_out=acc_sb[:, n : n + 1],
                )

                # ---- histogram prep ----
                # floor via magic-number rounding (no mod/divide on DVE ISA):
                # t1 = RNE(32x + 0.5 + 2^23) ; nf = t1 - (2^23+1) = floor(32x)
                # except ties (32x exactly integer k: even k -> k-1) and
                # 32x == 0 -> -1; corrected host-side (see _hist_fix).
                zmx, zmy, q1y = t_("zmx"), t_("zmy"), t_("q1y")
                nfx = tmp.tile([128, T], BF16, tag="nfx", name="nfx")[:]
                nfy = tmp.tile([128, T], BF16, tag="nfy", name="nfy")[:]
                hyb = tmp.tile([128, T], BF16, tag="hyb", name="hyb")[:]
                pyb = tmp.tile([128, T], BF16, tag="pyb", name="pyb")[:]
                wyb = tmp.tile([128, T], BF16, tag="wyb", name="wyb")[:]
                nc.vector.tensor_scalar(zmx, x2, 32.0, 0.5, OP.mult, OP.add)
                nc.vector.tensor_scalar(nfx, zmx, MAGIC, MAGIC + 1.0, OP.add, OP.subtract)
                nc.vector.tensor_scalar(zmy, y2, 32.0, 0.5, OP.mult, OP.add)
                nc.vector.tensor_scalar(nfy, zmy, MAGIC, MAGIC + 1.0, OP.add, OP.subtract)
                # y packed: hy = trickfloor(16y) in [-1..15], py = gy-2hy,
                # wy = 1+511*py in {1,512}; uy[m] = [hy==m]*wy packs bins
                # (2m, 2m+1) into one f32 psum slot (radix 512).
                nc.vector.tensor_scalar(q1y, y2, 16.0, 0.5, OP.mult, OP.add)
                nc.vector.tensor_scalar(hyb, q1y, MAGIC, MAGIC + 1.0, OP.add, OP.subtract)
                nc.vector.scalar_tensor_tensor(pyb, hyb, -2.0, nfy, OP.mult, OP.add)
                nc.vector.tensor_scalar(wyb, pyb, 511.0, 1.0, OP.mult, OP.add)

                for c in range(n_chunks):
                    ohx = ohp.tile([128, GRID * Tc], BF16, tag="ohx", name="ohx")
                    ohy = ohp.tile([128, (GRID // 2) * Tc], BF16, tag="ohy", name="ohy")
                    s = slice(c * Tc, (c + 1) * Tc)
                    for i in range(GRID):
                        nc.vector.tensor_scalar(
                            ohx[:, i * Tc : (i + 1) * Tc], nfx[:, s],
                            float(i), None, OP.is_equal,
                        )
                    for m in range(GRID // 2):
                        nc.vector.scalar_tensor_tensor(
                            ohy[:, m * Tc : (m + 1) * Tc], hyb[:, s],
                            float(m), wyb[:, s], OP.is_equal, OP.mult,
                        )
                    ohx_v = ohx.rearrange("p (i t) -> p t i", t=Tc)
                    ohy_v = ohy.rearrange("p (i t) -> p t i", t=Tc)
                    g = min(n // 2, n_grp - 1)
                    g_mms = (min((2 * g + 2) * 128 * T, NB)) // 128
                    g_first = (2 * g * 128 * T) // 128
                    for t in range(Tc):
                        nc.tensor.matmul(
                            ps_g[g][:], ohy_v[:, t], ohx_v[:, t],
                            start=(mm_i == g_first), stop=(mm_i == g_mms - 1),
                        )
                        mm_i += 1

            for g in range(n_grp):
                nc.vector.tensor_copy(hist_sb[:, g * GRID : (g + 1) * GRID], ps_g[g][:])
            nc.sync.dma_start(hist_d.ap(), hist_sb[:])
            nc.sync.dma_start(acc_d.ap(), acc_sb[:])

    nc.compile()
    return nc


_CACHE = {}
RUN_KW = {}
LAST_RESULT = None


def _get_program(NB, T, Tc):
    key = (NB, T, Tc)
    if key not in _CACHE:
        _CACHE[key] = build_nc(NB, T=T, Tc=Tc)
    return _CACHE[key]


def _trick_bins(v):
    """Replicate the device's magic-number binning exactly (f32 IEEE RNE)."""
    z05 = (v * np.float32(32.0) + np.float32(0.5)).astype(np.float32)  # exact
    t1 = (z05 + np.float32(MAGIC)).astype(np.float32)                  # RNE
    nf = (t1 - np.float32(MAGIC + 1.0)).astype(np.float32)             # exact
    return nf.astype(np.int64)


def _trick16(v):
    z05 = (v * np.float32(16.0) + np.float32(0.5)).astype(np.float32)
    t1 = (z05 + np.float32(MAGIC)).astype(np.float32)
    return (t1 - np.float32(MAGIC + 1.0)).astype(np.float32).astype(np.int64)


def _decode_hists(packed_list, targ, n_shard, pad, T):
    """Decode per-core packed histograms [16, 32*n_grp] (row m packs bins
    2m / 2m+1 at radix 512) into the true 32x32 histogram, moving the few
    fp-tie boxes (where the device trick-bin differs from floor) exactly."""
    n_grp = packed_list[0].shape[1] // GRID
    grp_boxes = 2 * 128 * T
    x, y = targ[:, 0], targ[:, 1]
    gx_t = _trick_bins(x)
    nfy = _trick_bins(y)
    hyb = _trick16(y)
    py = nfy - 2 * hyb
    gx_f = np.floor((x * np.float32(32.0)).astype(np.float32)).astype(np.int64)
    gy_f = np.floor((y * np.float32(32.0)).astype(np.float32)).astype(np.int64)
    clean = (gx_t == gx_f) & (hyb == gy_f // 2) & (py == gy_f % 2)
    hist = np.zeros((GRID, GRID), dtype=np.float64)
    for i in np.nonzero(~clean)[0]:
        c = i // n_shard
        pos = i - c * n_shard
        g = min(pos // grp_boxes, n_grp - 1)
        if 0 <= hyb[i] < 16 and 0 <= gx_t[i] < 32:
            packed_list[c][hyb[i], g * GRID + gx_t[i]] -= 1.0 + 511.0 * py[i]
        hist[gy_f[i], gx_f[i]] += 1.0
    for p in packed_list:
        for g in range(n_grp):
            P = p[:, g * GRID : (g + 1) * GRID]
            n1 = np.floor(P / 512.0)
            n0 = P - 512.0 * n1
            assert (n0 >= 0).all() and (n0 < 512).all() and (n1 >= 0).all(), "decode overflow"
            hist[0::2, :] += n0
            hist[1::2, :] += n1
    if pad:
        # pad box (x=y=0.5): 32v=16 tie->even => bin (15,15)
        hist[15, 15] -= pad * len(packed_list)
    return hist


def kernel(pred_boxes: np.ndarray, target_boxes: np.ndarray) -> np.ndarray:
    N = pred_boxes.shape[0]
    assert N % N_CORES == 0
    n_shard = N // N_CORES
    NB = NB_CORE if N == N_TOTAL else n_shard
    pad = NB - n_shard
    assert pad >= 0

    pred = np.ascontiguousarray(pred_boxes, dtype=np.float32)
    targ = np.ascontiguousarray(target_boxes, dtype=np.float32)

    in_maps = []
    for c in range(N_CORES):
        ps = pred[c * n_shard : (c + 1) * n_shard]
        ts = targ[c * n_shard : (c + 1) * n_shard]
        if pad:
            padrow = np.array(PAD_BOX, dtype=np.float32)[None].repeat(pad, 0)
            ps = np.concatenate([ps, padrow], 0)
            ts = np.concatenate([ts, padrow], 0)
        in_maps.append({"pred_boxes": ps, "target_boxes": ts})

    nc = _get_program(NB, 512, 256)
    res = bass_utils.run_bass_kernel_spmd(
        nc, in_maps, core_ids=list(range(N_CORES)), **RUN_KW
    )
    global LAST_RESULT
    LAST_RESULT = res

    base_sum = 0.0
    packed = []
    for r in res.results:
        base_sum += float(r["acc_out"].astype(np.float64).sum())
        packed.append(r["hist_out"].astype(np.float64))
    hist = _decode_hists(packed, targ, n_shard, pad, 512)
    assert hist.sum() == N, (hist.sum(), N)
    mean_base = base_sum / N
    max_h = hist.max()
    result = mean_base * (1.0 + ALPHA * (N / (GRID * GRID)) / max_h)
    return np.float32(result)



# revision 12
# speedup vs baseline: 1.3293x; 1.3293x over previous
"""DOSAConLoss Trainium2 kernel (v3).

result = mean(base) * (1 + ALPHA * (N/1024) / max_hist)
since sum(hist) == N exactly (every box center lands in one bin).

8-way data parallel over N. Host ships inputs as fp16 PLANAR [4, NB] per
core (x/y/w/h planes) — halves transfer bytes and makes per-plane SBUF
slices dense.

Per core, per tile of 128x512 boxes:
  - CIoU chain in f32 (bf16 where precision allows), atan via
      atan(a)-atan(b) = atan((w2*h1-w1*h2)/(h1*h2+w1*w2))
    (valid since both atans are in (0, pi/2)); one Arctan per tile.
  - base = (1-ciou)^3 / (w2*h2+1e-7) accumulated into acc_out[128, nt].
  - Histogram, radix-64 packed: per box, z = 16*v; bin-half j = floor(z)
    (16 bins), parity p = [frac >= .5]; weight wx = 1+63*px (x side),
    wy = 1+4095*py (y side), so the PE product carries the 2x2 subcell
    in base-64 digits (1, 64, 4096, 262144; counts < 64 per psum group).
    A custom DVE op (ONEHOT16W, registered at import into
    concourse.dve_ops) builds the weighted one-hot directly in t-major
    layout: out[t*16+j] = [0 <= z[t]-j < 1] * win[t] — one instruction
    per side per tile. t-major makes 8 consecutive box-columns a
    contiguous [128,128] block, so the PE does 64 stacked matmuls per
    tile (instead of 4096 tiny LDW-bound ones), accumulating all
    (t mod 8)-diagonal blocks into one psum[128,128] per tile.
Host decodes base-64 digits from the 8 diagonal 16x16 blocks per tile,
replicates the device's fp16 binning exactly in numpy, and moves boxes
whose fp16 bin differs from the f32 reference bin (~1%), making the
final histogram exact. Reciprocals via exp(-ln(x)) on ACT.
"""

import numpy as np

import concourse.bass as bass
import concourse.bacc as bacc
import concourse.mybir as mybir
import concourse.tile as tile
from concourse import bass_utils
from concourse import dve_ops as _dve_ops
from concourse.dve_spec import (
    AluOp as _AluOp, Bin as _Bin, Idx as _Idx, PageIdx as _PageIdx,
    Spec as _Spec, Src0 as _Src0, Src1 as _Src1, Zero as _Zero, One as _One,
    C0 as _C0, lower as _dve_lower, _has_src1,
)
from concourse.dve_uop import DveOpSpec as _DveOpSpec

# Keep Ln+Exp in one act table (natural_log_exp_and_others): hide them
# from the single-function sets so the chooser lands on the joint one.
_orig_get_act_tables = bacc.get_activation_tables


def _patched_get_act_tables(arch):
    t = {k: set(v) for k, v in _orig_get_act_tables(arch).items()}
    t.get("natural_log", set()).discard(mybir.ActivationFunctionType.Ln)
    t.get("exp_and_others", set()).discard(mybir.ActivationFunctionType.Exp)
    t.get("exp_and_friends", set()).discard(mybir.ActivationFunctionType.Exp)
    return t


bacc.get_activation_tables = _patched_get_act_tables


# ---- custom DVE op: t-major weighted 16-bin one-hot ----------------------
# out[p, t*16+j] = [0 <= in0[p,t]-j < 1] * in1[p,t]   (j = Idx - 16*page)
def _onehot_ref(in0, in1, s0, s1, imm2):
    P = in0.shape[0]
    z = in0.astype(np.float32).reshape(P, -1)
    w = in1.astype(np.float32).reshape(P, -1)
    S, T = int(s0), z.shape[1]
    out = np.zeros((P, T * S), np.float32)
    for t in range(T):
        t0 = z[:, t, None] - np.arange(S, dtype=np.float32)[None, :]
        m = (t0 >= 0.0) & (t0 < 1.0)
        out[:, t * S:(t + 1) * S] = m * w[:, t, None]
    return out


def _register_onehot_op():
    if "ONEHOT16W" in _dve_ops._SUB_OPCODE_FOR_NAME:
        return [op for op in _dve_ops.OPS if op.name == "ONEHOT16W"][0]
    _j = _Bin(_AluOp.SUBTRACT, _Idx, _PageIdx(_Zero, _C0))
    _t0 = _Bin(_AluOp.SUBTRACT, _Src0, _j)
    _m = _Bin(_AluOp.MULTIPLY, _t0 >= _Zero, _t0 < _One)
    spec = _Spec(body=_Bin(_AluOp.MULTIPLY, _m, _Src1), reference=_onehot_ref)
    row = max(_dve_ops._SUB_OPCODE_FOR_NAME.values()) + 1
    assert row < 0x20
    op = _dve_ops.DveOp("ONEHOT16W", spec, subdim=True, uops_sha={})
    _dve_ops.OPS.append(op)
    _dve_ops._SUB_OPCODE_FOR_NAME[op.name] = row
    _dve_ops.CUSTOM_DVE_SPECS[op.name] = spec
    for ver in ("v3", "v4"):
        _dve_ops._COMPILE_CACHE[(op.name, ver)] = _DveOpSpec(
            name=op.name, opcode=row, uops=_dve_lower(spec, ver=ver),
            rd1_en=_has_src1(spec),
        )
    return op


ONEHOT16W = _register_onehot_op()

F32 = mybir.dt.float32
BF16 = mybir.dt.bfloat16
FP16 = mybir.dt.float16
AF = mybir.ActivationFunctionType
OP = mybir.AluOpType

GRID = 32
ALPHA = 1.5
EPS = 1e-7
PI = float(np.pi)
MAGIC = float(2 ** 23)

N_CORES = 8
N_TOTAL = 4_000_000
T = 512
TILE_BOX = 128 * T
NB_CORE = 524_288
# pred==targ -> base ~1e-21; x=y=1.0 -> z=16 -> one-hot match fails, so
# pads never enter the device histogram (no radix-capacity risk)
PAD_BOX = (1.0, 1.0, 1.0, 1.0)

# ops routed to the (slow but otherwise idle) GPSIMD Pool engine
GPS_OPS = {"asum", "cw2", "ch2", "scw", "sch", "c24", "s12"}


def build_nc(NB, T=T, Tc=None):
    n_tiles = NB // (128 * T)
    assert NB == n_tiles * 128 * T and T % 8 == 0

    nc = bacc.Bacc("TRN2", target_bir_lowering=False, debug=False)
    pred_d = nc.dram_tensor("pred_boxes", [4, NB], FP16, kind="ExternalInput")
    targ_d = nc.dram_tensor("target_boxes", [4, NB], FP16, kind="ExternalInput")
    acc_d = nc.dram_tensor("acc_out", [128, n_tiles], F32, kind="ExternalOutput")
    hist_d = nc.dram_tensor("hist_out", [128, n_tiles * 128], F32, kind="ExternalOutput")

    pred_v = pred_d.ap().rearrange("c (n p t) -> n p c t", p=128, t=T)
    targ_v = targ_d.ap().rearrange("c (n p t) -> n p c t", p=128, t=T)

    def eng(name):
        return nc.gpsimd if name in GPS_OPS else nc.vector

    with tile.TileContext(nc) as tc:
        with (
            tc.tile_pool(name="inp", bufs=2) as inp,
            tc.tile_pool(name="tmp", bufs=2) as tmp,
            tc.tile_pool(name="ohp", bufs=2) as ohp,
            tc.tile_pool(name="cst", bufs=1) as cst,
            tc.tile_pool(name="psp", bufs=1, space="PSUM") as psp,
        ):
            bias_tiles = {}

            def bias_ap(val):
                if val not in bias_tiles:
                    t_ = cst.tile([128, 1], F32, name=f"bias{len(bias_tiles)}")
                    nc.vector.memset(t_[:], val)
                    bias_tiles[val] = t_[:]
                return bias_tiles[val]

            acc_sb = cst.tile([128, n_tiles], F32)
            hist_sb = cst.tile([128, n_tiles * 128], F32)
            ps = [psp.tile([128, 128], F32, name=f"ps{g}") for g in range(n_tiles)]

            NGEN = 14
            DEDICATED = {"a2t", "iou", "term1", "vv", "z"}
            gen_counter = [0]

            for n in range(n_tiles):
                pt = inp.tile([128, 4 * T], FP16, tag="pred")
                tt = inp.tile([128, 4 * T], FP16, tag="targ")
                p3 = pt.rearrange("p (c t) -> p c t", c=4)
                t3 = tt.rearrange("p (c t) -> p c t", c=4)
                nc.sync.dma_start(p3[:, :, :], pred_v[n])
                nc.sync.dma_start(t3[:, :, :], targ_v[n])
                x1, y1, w1, h1 = p3[:, 0], p3[:, 1], p3[:, 2], p3[:, 3]
                x2, y2, w2, h2 = t3[:, 0], t3[:, 1], t3[:, 2], t3[:, 3]

                def t_(tag, dt=F32):
                    if tag in DEDICATED:
                        return tmp.tile([128, T], dt, tag=tag, name=tag)[:]
                    i = gen_counter[0] % NGEN
                    gen_counter[0] += 1
                    return tmp.tile([128, T], dt, tag=f"g{i}", name=tag)[:]

                # ---- histogram prep + one-hots (early: feeds PE) ----
                # z = 16*v; fl = RNE(z); d = z - fl; p = [d < 0];
                # win = 1 + (W-1)*p   (x: W=64, y: W=4096)
                oh = {}
                for side, v_in, wmul in (("x", x2, 63.0), ("y", y2, 4095.0)):
                    zt = tmp.tile([128, T], F32, tag=f"z{side}", name=f"z{side}")
                    fl, d, pb, win = (t_(f"{side}{s}") for s in
                                      ("fl", "d", "pb", "win"))
                    nc.vector.tensor_scalar(zt[:], v_in, 16.0, None, OP.mult)
                    nc.vector.tensor_scalar(fl, zt[:], MAGIC, MAGIC, OP.add, OP.subtract)
                    nc.vector.scalar_tensor_tensor(d, fl, -1.0, zt[:], OP.mult, OP.add)
                    nc.vector.tensor_scalar(pb, d, 0.0, None, OP.is_lt)
                    nc.vector.tensor_scalar(win, pb, wmul, 1.0, OP.mult, OP.add)
                    oht = ohp.tile([128, T * 16], BF16, tag=f"oh{side}",
                                   name=f"oh{side}")
                    oh3 = oht.rearrange("p (t j) -> p t j", j=16)
                    z3 = zt[:].unsqueeze(2).broadcast_to([128, T, 16])
                    nc.vector._custom_dve(
                        ONEHOT16W, out=oh3[:, :, :], in0=z3,
                        in1=win.unsqueeze(2).broadcast_to([128, T, 16]),
                        s0=16.0,
                    )
                    oh[side] = oht

                n_mm = T // 8
                for k in range(n_mm):
                    nc.tensor.matmul(
                        ps[n][:],
                        oh["y"][:, 128 * k : 128 * k + 128],
                        oh["x"][:, 128 * k : 128 * k + 128],
                        start=(k == 0), stop=(k == n_mm - 1),
                    )
                nc.vector.tensor_copy(hist_sb[:, n * 128 : (n + 1) * 128], ps[n][:])

                # ---- CIoU chain ----
                dx, dy = t_("dx"), t_("dy")
                W, dW, H, dH = t_("W"), t_("dW"), t_("H"), t_("dH")
                nc.vector.tensor_tensor(dx, x1, x2, OP.subtract)
                nc.vector.tensor_tensor(dy, y1, y2, OP.subtract)
                nc.vector.tensor_tensor(W, w1, w2, OP.add)
                nc.vector.tensor_tensor(dW, w1, w2, OP.subtract)
                nc.vector.tensor_tensor(H, h1, h2, OP.add)
                nc.vector.tensor_tensor(dH, h1, h2, OP.subtract)
                a2t, a1t, asum = t_("a2t"), t_("a1t"), t_("asum")
                nc.vector.tensor_tensor(a2t, w2, h2, OP.mult)
                nc.vector.tensor_tensor(a1t, w1, h1, OP.mult)
                eng("asum").tensor_tensor(asum, a1t, a2t, OP.add)

                adx, ady, adW, adH = t_("adx"), t_("ady"), t_("adW"), t_("adH")
                nc.scalar.activation(adx, dx, AF.Abs, scale=2.0)
                nc.scalar.activation(ady, dy, AF.Abs, scale=2.0)
                nc.scalar.activation(adW, dW, AF.Abs)
                nc.scalar.activation(adH, dH, AF.Abs)
                mx, my = t_("mx"), t_("my")
                nc.vector.tensor_tensor(mx, adx, adW, OP.max)
                nc.vector.tensor_tensor(my, ady, adH, OP.max)

                iw4, ih4, ihc, inter4 = t_("iw4"), t_("ih4"), t_("ihc"), t_("inter4")
                nc.vector.scalar_tensor_tensor(iw4, mx, -1.0, W, OP.mult, OP.add)
                nc.vector.scalar_tensor_tensor(ih4, my, -1.0, H, OP.mult, OP.add)
                nc.scalar.activation(ihc, ih4, AF.Relu)
                nc.vector.scalar_tensor_tensor(inter4, iw4, 0.0, ihc, OP.max, OP.mult)
                u4 = t_("u4")
                nc.vector.scalar_tensor_tensor(u4, inter4, -0.25, asum, OP.mult, OP.add)
                lnu, r_u = t_("lnu"), t_("r_u")
                nc.scalar.activation(lnu, u4, AF.Ln, scale=4.0, bias=bias_ap(4 * EPS))
                nc.scalar.activation(r_u, lnu, AF.Exp, scale=-1.0)
                iou = t_("iou")
                nc.vector.tensor_tensor(iou, inter4, r_u, OP.mult)

                cw2, ch2 = t_("cw2"), t_("ch2")
                eng("cw2").tensor_tensor(cw2, W, mx, OP.add)
                eng("ch2").tensor_tensor(ch2, H, my, OP.add)
                scw, sch, sdx, sdy = t_("scw"), t_("sch"), t_("sdx"), t_("sdy")
                eng("scw").tensor_tensor(scw, cw2, cw2, OP.mult)
                eng("sch").tensor_tensor(sch, ch2, ch2, OP.mult)
                nc.scalar.activation(sdx, dx, AF.Square, scale=2.0)
                nc.scalar.activation(sdy, dy, AF.Square, scale=2.0)
                c24, rho4 = t_("c24"), t_("rho4")
                eng("c24").tensor_tensor(c24, scw, sch, OP.add)
                nc.vector.tensor_tensor(rho4, sdx, sdy, OP.add)
                lnc, r_c = t_("lnc"), t_("r_c")
                nc.scalar.activation(lnc, c24, AF.Ln, bias=bias_ap(4 * EPS))
                nc.scalar.activation(r_c, lnc, AF.Exp, scale=-1.0)
                term1 = t_("term1")
                nc.vector.tensor_tensor(term1, rho4, r_c, OP.mult)

                # atan(w2/h2)-atan(w1/h1) = atan((w2*h1-w1*h2)/(h1*h2+w1*w2))
                c1, c2, d1, d2 = t_("c1"), t_("c2"), t_("d1"), t_("d2")
                nc.vector.tensor_tensor(c1, w2, h1, OP.mult)
                nc.vector.tensor_tensor(c2, w1, h2, OP.mult)
                nc.vector.tensor_tensor(d1, h1, h2, OP.mult)
                nc.vector.tensor_tensor(d2, w1, w2, OP.mult)
                numq, den = t_("numq"), t_("den")
                nc.vector.tensor_tensor(numq, c1, c2, OP.subtract)
                nc.vector.tensor_tensor(den, d1, d2, OP.add)
                lnd, r_d = t_("lnd"), t_("r_d")
                nc.scalar.activation(lnd, den, AF.Ln, bias=bias_ap(1e-30))
                nc.scalar.activation(r_d, lnd, AF.Exp, scale=-1.0)
                lnsw, sw = t_("lnsw"), t_("sw")
                nc.scalar.activation(lnsw, a2t, AF.Ln, bias=bias_ap(1e-7))
                nc.scalar.activation(sw, lnsw, AF.Exp, scale=-1.0)
                q, at = t_("q"), t_("at")
                nc.vector.tensor_tensor(q, numq, r_d, OP.mult)
                nc.scalar.activation(at, q, AF.Arctan)
                vv = t_("vv")
                nc.scalar.activation(vv, at, AF.Square, scale=2.0 / PI)

                den0 = t_("den0")
                nc.vector.tensor_tensor(den0, vv, iou, OP.subtract)
                lnden, rden, v2 = t_("lnden"), t_("rden"), t_("v2")
                nc.scalar.activation(lnden, den0, AF.Ln, bias=bias_ap(1.0 + EPS))
                nc.scalar.activation(rden, lnden, AF.Exp, scale=-1.0)
                nc.scalar.activation(v2, vv, AF.Square)
                term2, s12, z = t_("term2"), t_("s12"), t_("z")
                nc.vector.tensor_tensor(term2, v2, rden, OP.mult)
                eng("s12").tensor_tensor(s12, term1, term2, OP.add)
                nc.vector.scalar_tensor_tensor(z, iou, -1.0, s12, OP.mult, OP.add)

                om2, om3 = t_("om2"), t_("om3")
                nc.scalar.activation(om2, z, AF.Square, bias=bias_ap(1.0))
                nc.vector.scalar_tensor_tensor(om3, z, 1.0, om2, OP.add, OP.mult)
                nc.vector.scalar_tensor_tensor(
                    t_("baset"), om3, 0.0, sw, OP.add, OP.mult,
                    accum_out=acc_sb[:, n : n + 1],
                )

            nc.sync.dma_start(hist_d.ap(), hist_sb[:])
            nc.sync.dma_start(acc_d.ap(), acc_sb[:])

    nc.compile()
    return nc


_CACHE = {}
RUN_KW = {}
LAST_RESULT = None


def _get_program(NB, T_=T, Tc=None):
    key = (NB, T_)
    if key not in _CACHE:
        _CACHE[key] = build_nc(NB, T=T_)
    return _CACHE[key]


def _dev_bins(v16):
    """Replicate the device binning exactly from the fp16 plane.
    Device: z = 16*v (exact f32); j = floor(z) matched iff 0<=j<16;
    parity p = [z - RNE(z) < 0]. Returns (j, p)."""
    z = np.float32(16.0) * v16.astype(np.float32)
    fl = ((z + np.float32(MAGIC)) - np.float32(MAGIC)).astype(np.float32)
    p = ((z - fl) < 0).astype(np.int64)
    return np.floor(z).astype(np.int64), p


def _true_bins(v32):
    return np.clip((v32 * np.float32(GRID)).astype(np.int32), 0, GRID - 1).astype(np.int64)


def _decode_hists(raw_list, n_tiles):
    """Per-core [128, n_tiles*128] psum dumps -> exact device histogram.
    Diagonal 16x16 blocks hold base-64 digit-packed counts."""
    hist = np.zeros((GRID, GRID), dtype=np.float64)
    for raw in raw_list:
        R = raw.reshape(128, n_tiles, 128).astype(np.float64)
        for g in range(n_tiles):
            P = R[:, g, :]
            for ti in range(8):
                D = P[16 * ti : 16 * ti + 16, 16 * ti : 16 * ti + 16]
                d3 = np.floor(D / 262144.0)
                r = D - d3 * 262144.0
                d2 = np.floor(r / 4096.0)
                r -= d2 * 4096.0
                d1 = np.floor(r / 64.0)
                d0 = r - d1 * 64.0
                for dd in (d0, d1, d2, d3):
                    assert (dd >= 0).all() and (dd < 64).all(), "radix overflow"
                hist[0::2, 0::2] += d0
                hist[0::2, 1::2] += d1
                hist[1::2, 0::2] += d2
                hist[1::2, 1::2] += d3
    return hist


def kernel(pred_boxes: np.ndarray, target_boxes: np.ndarray) -> np.ndarray:
    N = pred_boxes.shape[0]
    assert N % N_CORES == 0
    n_shard = N // N_CORES
    NB = ((n_shard + TILE_BOX - 1) // TILE_BOX) * TILE_BOX
    n_tiles = NB // TILE_BOX
    pad = NB - n_shard

    pred = np.asarray(pred_boxes, dtype=np.float32)
    targ = np.asarray(target_boxes, dtype=np.float32)

    padrow = np.array(PAD_BOX, dtype=np.float16)
    in_maps = []
    targ16 = []
    for c in range(N_CORES):
        pm = np.empty((4, NB), dtype=np.float16)
        tm = np.empty((4, NB), dtype=np.float16)
        pm[:, :n_shard] = pred[c * n_shard : (c + 1) * n_shard].T
        tm[:, :n_shard] = targ[c * n_shard : (c + 1) * n_shard].T
        if pad:
            pm[:, n_shard:] = padrow[:, None]
            tm[:, n_shard:] = padrow[:, None]
        in_maps.append({"pred_boxes": pm, "target_boxes": tm})
        targ16.append(tm)

    nc = _get_program(NB)
    res = bass_utils.run_bass_kernel_spmd(
        nc, in_maps, core_ids=list(range(N_CORES)), **RUN_KW
    )
    global LAST_RESULT
    LAST_RESULT = res

    base_sum = 0.0
    raws = []
    for r in res.results:
        base_sum += float(r["acc_out"].astype(np.float64).sum())
        raws.append(r["hist_out"])
    hist = _decode_hists(raws, n_tiles)

    # exact fixup: move boxes whose device (fp16) bin differs from the
    # f32 reference bin; drop pads; add device-dropped boxes (j>15)
    x16 = np.concatenate([t[0] for t in targ16])
    y16 = np.concatenate([t[1] for t in targ16])
    fx, px = _dev_bins(x16)
    fy, py = _dev_bins(y16)
    bx_dev = 2 * fx + px
    by_dev = 2 * fy + py
    counted = (fx >= 0) & (fx < 16) & (fy >= 0) & (fy < 16)
    is_real = np.zeros(NB * N_CORES, dtype=bool)
    for c in range(N_CORES):
        is_real[c * NB : c * NB + n_shard] = True
    gx_t = _true_bins(np.concatenate(
        [targ[c * n_shard : (c + 1) * n_shard, 0] for c in range(N_CORES)]))
    gy_t = _true_bins(np.concatenate(
        [targ[c * n_shard : (c + 1) * n_shard, 1] for c in range(N_CORES)]))
    bx_r = bx_dev[is_real]
    by_r = by_dev[is_real]
    cnt_r = counted[is_real]
    ok = cnt_r & (bx_r == gx_t) & (by_r == gy_t)
    sub_x = np.concatenate([bx_r[cnt_r & ~ok], bx_dev[~is_real & counted]])
    sub_y = np.concatenate([by_r[cnt_r & ~ok], by_dev[~is_real & counted]])
    np.subtract.at(hist, (sub_y, sub_x), 1.0)
    np.add.at(hist, (gy_t[~ok], gx_t[~ok]), 1.0)
    assert hist.sum() == N, (hist.sum(), N)

    mean_base = base_sum / N
    max_h = hist.max()
    result = mean_base * (1.0 + ALPHA * (N / (GRID * GRID)) / max_h)
    return np.float32(result)


# revision 17
# speedup vs baseline: 1.3818x; 1.0395x over previous
"""DOSAConLoss Trainium2 kernel (v3).

result = mean(base) * (1 + ALPHA * (N/1024) / max_hist)
since sum(hist) == N exactly (every box center lands in one bin).

8-way data parallel over N. Host ships inputs as fp16 PLANAR [4, NB] per
core (x/y/w/h planes) — halves transfer bytes and makes per-plane SBUF
slices dense.

Per core, per tile of 128x512 boxes:
  - CIoU chain in f32 (bf16 where precision allows), atan via
      atan(a)-atan(b) = atan((w2*h1-w1*h2)/(h1*h2+w1*w2))
    (valid since both atans are in (0, pi/2)); one Arctan per tile.
  - base = (1-ciou)^3 / (w2*h2+1e-7) accumulated into acc_out[128, nt].
  - Histogram, radix-64 packed: per box, z = 16*v; bin-half j = floor(z)
    (16 bins), parity p = [frac >= .5]; weight wx = 1+63*px (x side),
    wy = 1+4095*py (y side), so the PE product carries the 2x2 subcell
    in base-64 digits (1, 64, 4096, 262144; counts < 64 per psum group).
    A custom DVE op (ONEHOT16W, registered at import into
    concourse.dve_ops) builds the weighted one-hot directly in t-major
    layout: out[t*16+j] = [0 <= z[t]-j < 1] * win[t] — one instruction
    per side per tile. t-major makes 8 consecutive box-columns a
    contiguous [128,128] block, so the PE does 64 stacked matmuls per
    tile (instead of 4096 tiny LDW-bound ones), accumulating all
    (t mod 8)-diagonal blocks into one psum[128,128] per tile.
Host decodes base-64 digits from the 8 diagonal 16x16 blocks per tile,
replicates the device's fp16 binning exactly in numpy, and moves boxes
whose fp16 bin differs from the f32 reference bin (~1%), making the
final histogram exact. Reciprocals via exp(-ln(x)) on ACT.
"""

import numpy as np

import concourse.bass as bass
import concourse.bacc as bacc
import concourse.mybir as mybir
import concourse.tile as tile
from concourse import bass_utils
from concourse import dve_ops as _dve_ops
from concourse.dve_spec import (
    AluOp as _AluOp, Bin as _Bin, Idx as _Idx, PageIdx as _PageIdx,
    Spec as _Spec, Src0 as _Src0, Src1 as _Src1, Zero as _Zero, One as _One,
    C0 as _C0, lower as _dve_lower, _has_src1,
)
from concourse.dve_uop import DveOpSpec as _DveOpSpec

# Keep Ln+Exp in one act table (natural_log_exp_and_others): hide them
# from the single-function sets so the chooser lands on the joint one.
_orig_get_act_tables = bacc.get_activation_tables


def _patched_get_act_tables(arch):
    t = {k: set(v) for k, v in _orig_get_act_tables(arch).items()}
    t.get("natural_log", set()).discard(mybir.ActivationFunctionType.Ln)
    t.get("exp_and_others", set()).discard(mybir.ActivationFunctionType.Exp)
    t.get("exp_and_friends", set()).discard(mybir.ActivationFunctionType.Exp)
    return t


bacc.get_activation_tables = _patched_get_act_tables


# ---- custom DVE op: t-major weighted 16-bin one-hot ----------------------
# out[p, t*16+j] = [0 <= in0[p,t]-j < 1] * in1[p,t]   (j = Idx - 16*page)
def _onehot_ref(in0, in1, s0, s1, imm2):
    P = in0.shape[0]
    z = in0.astype(np.float32).reshape(P, -1)
    w = in1.astype(np.float32).reshape(P, -1)
    S, T = int(s0), z.shape[1]
    out = np.zeros((P, T * S), np.float32)
    for t in range(T):
        t0 = z[:, t, None] - np.arange(S, dtype=np.float32)[None, :]
        m = (t0 >= 0.0) & (t0 < 1.0)
        out[:, t * S:(t + 1) * S] = m * w[:, t, None]
    return out


def _register_onehot_op():
    if "ONEHOT16W" in _dve_ops._SUB_OPCODE_FOR_NAME:
        return [op for op in _dve_ops.OPS if op.name == "ONEHOT16W"][0]
    _j = _Bin(_AluOp.SUBTRACT, _Idx, _PageIdx(_Zero, _C0))
    _t0 = _Bin(_AluOp.SUBTRACT, _Src0, _j)
    _m = _Bin(_AluOp.MULTIPLY, _t0 >= _Zero, _t0 < _One)
    spec = _Spec(body=_Bin(_AluOp.MULTIPLY, _m, _Src1), reference=_onehot_ref)
    row = max(_dve_ops._SUB_OPCODE_FOR_NAME.values()) + 1
    assert row < 0x20
    op = _dve_ops.DveOp("ONEHOT16W", spec, subdim=True, uops_sha={})
    _dve_ops.OPS.append(op)
    _dve_ops._SUB_OPCODE_FOR_NAME[op.name] = row
    _dve_ops.CUSTOM_DVE_SPECS[op.name] = spec
    for ver in ("v3", "v4"):
        _dve_ops._COMPILE_CACHE[(op.name, ver)] = _DveOpSpec(
            name=op.name, opcode=row, uops=_dve_lower(spec, ver=ver),
            rd1_en=_has_src1(spec),
        )
    return op


ONEHOT16W = _register_onehot_op()

F32 = mybir.dt.float32
BF16 = mybir.dt.bfloat16
FP16 = mybir.dt.float16
AF = mybir.ActivationFunctionType
OP = mybir.AluOpType

GRID = 32
ALPHA = 1.5
EPS = 1e-7
PI = float(np.pi)
MAGIC = float(2 ** 23)

N_CORES = 8
N_TOTAL = 4_000_000
T = 512
TILE_BOX = 128 * T
NB_CORE = 524_288
# pred==targ -> base ~1e-21; x=y=1.0 -> z=16 -> one-hot match fails, so
# pads never enter the device histogram (no radix-capacity risk)
PAD_BOX = (1.0, 1.0, 1.0, 1.0)

# ops routed to the (slow but otherwise idle) GPSIMD Pool engine
GPS_OPS = {"asum", "cw2", "ch2", "scw", "sch", "c24", "iou"}


def build_nc(NB, T=T, Tc=None):
    n_tiles = NB // (128 * T)
    assert NB == n_tiles * 128 * T and T % 8 == 0

    nc = bacc.Bacc("TRN2", target_bir_lowering=False, debug=False)
    pred_d = nc.dram_tensor("pred_boxes", [4, NB], FP16, kind="ExternalInput")
    targ_d = nc.dram_tensor("target_boxes", [4, NB], FP16, kind="ExternalInput")
    acc_d = nc.dram_tensor("acc_out", [128, n_tiles], F32, kind="ExternalOutput")
    hist_d = nc.dram_tensor("hist_out", [128, n_tiles * 128], F32, kind="ExternalOutput")

    pred_v = pred_d.ap().rearrange("c (n p t) -> n p c t", p=128, t=T)
    targ_v = targ_d.ap().rearrange("c (n p t) -> n p c t", p=128, t=T)

    def eng(name):
        return nc.gpsimd if name in GPS_OPS else nc.vector

    with tile.TileContext(nc) as tc:
        with (
            tc.tile_pool(name="inp", bufs=2) as inp,
            tc.tile_pool(name="tmp", bufs=2) as tmp,
            tc.tile_pool(name="ohp", bufs=2) as ohp,
            tc.tile_pool(name="cst", bufs=1) as cst,
            tc.tile_pool(name="psp", bufs=1, space="PSUM") as psp,
        ):
            bias_tiles = {}

            def bias_ap(val):
                if val not in bias_tiles:
                    t_ = cst.tile([128, 1], F32, name=f"bias{len(bias_tiles)}")
                    nc.vector.memset(t_[:], val)
                    bias_tiles[val] = t_[:]
                return bias_tiles[val]

            acc_sb = cst.tile([128, n_tiles], F32)
            hist_sb = cst.tile([128, n_tiles * 128], F32)
            ps = [psp.tile([128, 128], F32, name=f"ps{g}") for g in range(n_tiles)]

            NGEN = 14
            DEDICATED = {"a2t", "iou", "term1", "vv", "z"}
            gen_counter = [0]

            for n in range(n_tiles):
                pt = inp.tile([128, 4 * T], FP16, tag="pred")
                tt = inp.tile([128, 4 * T], FP16, tag="targ")
                p3 = pt.rearrange("p (c t) -> p c t", c=4)
                t3 = tt.rearrange("p (c t) -> p c t", c=4)
                nc.sync.dma_start(p3[:, :, :], pred_v[n])
                nc.sync.dma_start(t3[:, :, :], targ_v[n])
                x1, y1, w1, h1 = p3[:, 0], p3[:, 1], p3[:, 2], p3[:, 3]
                x2, y2, w2, h2 = t3[:, 0], t3[:, 1], t3[:, 2], t3[:, 3]

                def t_(tag, dt=F32):
                    if tag in DEDICATED:
                        return tmp.tile([128, T], dt, tag=tag, name=tag)[:]
                    i = gen_counter[0] % NGEN
                    gen_counter[0] += 1
                    return tmp.tile([128, T], dt, tag=f"g{i}", name=tag)[:]

                # ---- histogram prep + one-hots (early: feeds PE) ----
                # x (moving, m-major): fx = trickfloor(16x) via magic-RNE,
                #   parity px = [16x+0.5-fx >= 1]; pure one-hot [fx==j].
                # y (stationary, t-major custom op): zy = 16y; match
                #   j = floor(zy); parity py = [zy - RNE(zy) < 0]; the op
                #   carries the combined weight wc=(1+63px)(1+4095py).
                zx5 = t_("zx5")
                fxb = tmp.tile([128, T], BF16, tag="fxb", name="fxb")[:]
                pxb = tmp.tile([128, T], BF16, tag="pxb", name="pxb")[:]
                pyb = tmp.tile([128, T], BF16, tag="pyb", name="pyb")[:]
                txw = tmp.tile([128, T], BF16, tag="txw", name="txw")[:]
                tyw = tmp.tile([128, T], BF16, tag="tyw", name="tyw")[:]
                wcb = tmp.tile([128, T], BF16, tag="wcb", name="wcb")[:]
                frx = t_("frx")
                nc.scalar.activation(zx5, x2, AF.Identity, scale=16.0,
                                     bias=bias_ap(0.5))
                nc.vector.tensor_scalar(fxb, zx5, MAGIC, MAGIC + 1.0, OP.add, OP.subtract)
                nc.vector.scalar_tensor_tensor(frx, fxb, -1.0, zx5, OP.mult, OP.add)
                nc.vector.tensor_scalar(pxb, frx, 1.0, None, OP.is_ge)
                zy = tmp.tile([128, T], F32, tag="zy", name="zy")
                fly, dyt = t_("fly"), t_("dyt")
                nc.scalar.activation(zy[:], y2, AF.Identity, scale=16.0)
                nc.vector.tensor_scalar(fly, zy[:], MAGIC, MAGIC, OP.add, OP.subtract)
                nc.vector.scalar_tensor_tensor(dyt, fly, -1.0, zy[:], OP.mult, OP.add)
                nc.vector.tensor_scalar(pyb, dyt, 0.0, None, OP.is_lt)
                nc.vector.tensor_scalar(txw, pxb, 63.0, 1.0, OP.mult, OP.add)
                nc.vector.tensor_scalar(tyw, pyb, 4095.0, 1.0, OP.mult, OP.add)
                nc.vector.tensor_tensor(wcb, txw, tyw, OP.mult)

                ohx = ohp.tile([128, 16 * T], BF16, tag="ohx", name="ohx")
                ohy = ohp.tile([128, T * 16], BF16, tag="ohy", name="ohy")
                for j in range(16):
                    nc.vector.tensor_scalar(
                        ohx[:, j * T : (j + 1) * T], fxb, float(j), None, OP.is_equal
                    )
                oy3 = ohy.rearrange("p (t j) -> p t j", j=16)
                nc.vector._custom_dve(
                    ONEHOT16W, out=oy3[:, :, :],
                    in0=zy[:].unsqueeze(2).broadcast_to([128, T, 16]),
                    in1=wcb.unsqueeze(2).broadcast_to([128, T, 16]),
                    s0=16.0,
                )
                ox3 = ohx.rearrange("p (j t) -> p t j", t=T)
                n_mm = T // 8
                for k in range(n_mm):
                    nc.tensor.matmul(
                        ps[n][:],
                        ohy[:, 128 * k : 128 * k + 128],
                        ox3[:, 8 * k : 8 * k + 8],
                        start=(k == 0), stop=(k == n_mm - 1),
                    )
                nc.vector.tensor_copy(hist_sb[:, n * 128 : (n + 1) * 128], ps[n][:])

                # ---- CIoU chain ----
                dx, dy = t_("dx"), t_("dy")
                W, dW, H, dH = t_("W"), t_("dW"), t_("H"), t_("dH")
                nc.vector.tensor_tensor(dx, x1, x2, OP.subtract)
                nc.vector.tensor_tensor(dy, y1, y2, OP.subtract)
                nc.vector.tensor_tensor(W, w1, w2, OP.add)
                nc.vector.tensor_tensor(dW, w1, w2, OP.subtract)
                nc.vector.tensor_tensor(H, h1, h2, OP.add)
                nc.vector.tensor_tensor(dH, h1, h2, OP.subtract)
                a2t, a1t, asum = t_("a2t"), t_("a1t"), t_("asum")
                nc.vector.tensor_tensor(a2t, w2, h2, OP.mult)
                nc.vector.tensor_tensor(a1t, w1, h1, OP.mult)
                eng("asum").tensor_tensor(asum, a1t, a2t, OP.add)

                adx, ady, adW, adH = t_("adx"), t_("ady"), t_("adW"), t_("adH")
                nc.scalar.activation(adx, dx, AF.Abs, scale=2.0)
                nc.scalar.activation(ady, dy, AF.Abs, scale=2.0)
                nc.scalar.activation(adW, dW, AF.Abs)
                nc.scalar.activation(adH, dH, AF.Abs)
                mx, my = t_("mx"), t_("my")
                nc.vector.tensor_tensor(mx, adx, adW, OP.max)
                nc.vector.tensor_tensor(my, ady, adH, OP.max)

                iw4, ih4, ihc, inter4 = t_("iw4"), t_("ih4"), t_("ihc"), t_("inter4")
                nc.vector.scalar_tensor_tensor(iw4, mx, -1.0, W, OP.mult, OP.add)
                nc.vector.scalar_tensor_tensor(ih4, my, -1.0, H, OP.mult, OP.add)
                nc.scalar.activation(ihc, ih4, AF.Relu)
                nc.vector.scalar_tensor_tensor(inter4, iw4, 0.0, ihc, OP.max, OP.mult)
                u4 = t_("u4")
                nc.vector.scalar_tensor_tensor(u4, inter4, -0.25, asum, OP.mult, OP.add)
                lnu, r_u = t_("lnu"), t_("r_u")
                nc.scalar.activation(lnu, u4, AF.Ln, scale=4.0, bias=bias_ap(4 * EPS))
                nc.scalar.activation(r_u, lnu, AF.Exp, scale=-1.0)
                iou = t_("iou")
                nc.vector.tensor_tensor(iou, inter4, r_u, OP.mult)

                def tb(tag):
                    return tmp.tile([128, T], BF16, tag=f"b_{tag}", name=tag)[:]

                cw2, ch2 = t_("cw2"), t_("ch2")
                eng("cw2").tensor_tensor(cw2, W, mx, OP.add)
                eng("ch2").tensor_tensor(ch2, H, my, OP.add)
                scw, sch = t_("scw"), t_("sch")
                sdx, sdy = tb("sdx"), tb("sdy")
                eng("scw").tensor_tensor(scw, cw2, cw2, OP.mult)
                eng("sch").tensor_tensor(sch, ch2, ch2, OP.mult)
                nc.scalar.activation(sdx, dx, AF.Square, scale=2.0)
                nc.scalar.activation(sdy, dy, AF.Square, scale=2.0)
                c24 = t_("c24")
                rho4 = tb("rho4")
                eng("c24").tensor_tensor(c24, scw, sch, OP.add)
                nc.vector.tensor_tensor(rho4, sdx, sdy, OP.add)
                lnc = t_("lnc")
                r_c = tb("r_c")
                nc.scalar.activation(lnc, c24, AF.Ln, bias=bias_ap(4 * EPS))
                nc.scalar.activation(r_c, lnc, AF.Exp, scale=-1.0)
                term1 = tb("term1")
                nc.vector.tensor_tensor(term1, rho4, r_c, OP.mult)

                # atan(w2/h2)-atan(w1/h1) = atan((w2*h1-w1*h2)/(h1*h2+w1*w2))
                c1, c2, d1, d2 = tb("c1"), tb("c2"), tb("d1"), tb("d2")
                nc.vector.tensor_tensor(c1, w2, h1, OP.mult)
                nc.vector.tensor_tensor(c2, w1, h2, OP.mult)
                nc.vector.tensor_tensor(d1, h1, h2, OP.mult)
                nc.vector.tensor_tensor(d2, w1, w2, OP.mult)
                numq, den = tb("numq"), tb("den")
                nc.vector.tensor_tensor(numq, c1, c2, OP.subtract)
                nc.vector.tensor_tensor(den, d1, d2, OP.add)
                lnd = t_("lnd")
                r_d = tb("r_d")
                nc.scalar.activation(lnd, den, AF.Ln, bias=bias_ap(1e-30))
                nc.scalar.activation(r_d, lnd, AF.Exp, scale=-1.0)
                lnsw, sw = t_("lnsw"), t_("sw")
                nc.scalar.activation(lnsw, a2t, AF.Ln, bias=bias_ap(1e-7))
                nc.scalar.activation(sw, lnsw, AF.Exp, scale=-1.0)
                q = tb("q")
                at = t_("at")
                nc.vector.tensor_tensor(q, numq, r_d, OP.mult)
                nc.scalar.activation(at, q, AF.Arctan)
                vv = t_("vv")
                nc.scalar.activation(vv, at, AF.Square, scale=2.0 / PI)

                den0 = t_("den0")
                nc.vector.tensor_tensor(den0, vv, iou, OP.subtract)
                lnden = t_("lnden")
                rden, v2 = tb("rden"), tb("v2")
                nc.scalar.activation(lnden, den0, AF.Ln, bias=bias_ap(1.0 + EPS))
                nc.scalar.activation(rden, lnden, AF.Exp, scale=-1.0)
                nc.scalar.activation(v2, vv, AF.Square)
                term2, s12 = tb("term2"), tb("s12")
                z = t_("z")
                nc.vector.tensor_tensor(term2, v2, rden, OP.mult)
                nc.vector.tensor_tensor(s12, term1, term2, OP.add)
                nc.vector.scalar_tensor_tensor(z, iou, -1.0, s12, OP.mult, OP.add)

                om2, om3 = t_("om2"), t_("om3")
                nc.scalar.activation(om2, z, AF.Square, bias=bias_ap(1.0))
                nc.vector.scalar_tensor_tensor(om3, z, 1.0, om2, OP.add, OP.mult)
                nc.vector.scalar_tensor_tensor(
                    t_("baset"), om3, 0.0, sw, OP.add, OP.mult,
                    accum_out=acc_sb[:, n : n + 1],
                )

            nc.sync.dma_start(hist_d.ap(), hist_sb[:])
            nc.sync.dma_start(acc_d.ap(), acc_sb[:])

    nc.compile()
    return nc


_CACHE = {}
RUN_KW = {}
LAST_RESULT = None


def _get_program(NB, T_=T, Tc=None):
    key = (NB, T_)
    if key not in _CACHE:
        _CACHE[key] = build_nc(NB, T=T_)
    return _CACHE[key]


def _dev_bins_x(v16):
    """x-side device binning: z5 = 16v+0.5; fx = RNE(z5+M)-(M+1);
    px = [z5 - fx >= 1]. Counted iff 0 <= fx < 16."""
    z5 = (np.float32(16.0) * v16.astype(np.float32) + np.float32(0.5)).astype(np.float32)
    fx = ((z5 + np.float32(MAGIC)).astype(np.float32)
          - np.float32(MAGIC + 1.0)).astype(np.float32)
    px = ((z5 - fx) >= np.float32(1.0)).astype(np.int64)
    return fx.astype(np.int64), px


def _dev_bins_y(v16):
    """y-side device binning (custom op): z = 16v; j = floor(z) matched
    iff 0<=j<16; parity p = [z - RNE(z) < 0]."""
    z = np.float32(16.0) * v16.astype(np.float32)
    fl = ((z + np.float32(MAGIC)) - np.float32(MAGIC)).astype(np.float32)
    p = ((z - fl) < 0).astype(np.int64)
    return np.floor(z).astype(np.int64), p


def _true_bins(v32):
    return np.clip((v32 * np.float32(GRID)).astype(np.int32), 0, GRID - 1).astype(np.int64)


def _decode_hists(raw_list, n_tiles):
    """Per-core [128, n_tiles*128] psum dumps -> exact device histogram.
    Diagonal 16x16 blocks hold base-64 digit-packed counts."""
    hist = np.zeros((GRID, GRID), dtype=np.float64)
    for raw in raw_list:
        R = raw.reshape(128, n_tiles, 128).astype(np.float64)
        for g in range(n_tiles):
            P = R[:, g, :]
            for ti in range(8):
                D = P[16 * ti : 16 * ti + 16, 16 * ti : 16 * ti + 16]
                d3 = np.floor(D / 262144.0)
                r = D - d3 * 262144.0
                d2 = np.floor(r / 4096.0)
                r -= d2 * 4096.0
                d1 = np.floor(r / 64.0)
                d0 = r - d1 * 64.0
                for dd in (d0, d1, d2, d3):
                    assert (dd >= 0).all() and (dd < 64).all(), "radix overflow"
                hist[0::2, 0::2] += d0
                hist[0::2, 1::2] += d1
                hist[1::2, 0::2] += d2
                hist[1::2, 1::2] += d3
    return hist


def kernel(pred_boxes: np.ndarray, target_boxes: np.ndarray) -> np.ndarray:
    N = pred_boxes.shape[0]
    assert N % N_CORES == 0
    n_shard = N // N_CORES
    NB = ((n_shard + TILE_BOX - 1) // TILE_BOX) * TILE_BOX
    n_tiles = NB // TILE_BOX
    pad = NB - n_shard

    pred = np.asarray(pred_boxes, dtype=np.float32)
    targ = np.asarray(target_boxes, dtype=np.float32)

    padrow = np.array(PAD_BOX, dtype=np.float16)
    in_maps = []
    targ16 = []
    for c in range(N_CORES):
        pm = np.empty((4, NB), dtype=np.float16)
        tm = np.empty((4, NB), dtype=np.float16)
        pm[:, :n_shard] = pred[c * n_shard : (c + 1) * n_shard].T
        tm[:, :n_shard] = targ[c * n_shard : (c + 1) * n_shard].T
        if pad:
            pm[:, n_shard:] = padrow[:, None]
            tm[:, n_shard:] = padrow[:, None]
        in_maps.append({"pred_boxes": pm, "target_boxes": tm})
        targ16.append(tm)

    nc = _get_program(NB)
    res = bass_utils.run_bass_kernel_spmd(
        nc, in_maps, core_ids=list(range(N_CORES)), **RUN_KW
    )
    global LAST_RESULT
    LAST_RESULT = res

    base_sum = 0.0
    raws = []
    for r in res.results:
        base_sum += float(r["acc_out"].astype(np.float64).sum())
        raws.append(r["hist_out"])
    hist = _decode_hists(raws, n_tiles)

    # exact fixup: move boxes whose device (fp16) bin differs from the
    # f32 reference bin; drop pads; add device-dropped boxes (j>15)
    x16 = np.concatenate([t[0] for t in targ16])
    y16 = np.concatenate([t[1] for t in targ16])
    fx, px = _dev_bins_x(x16)
    fy, py = _dev_bins_y(y16)
    bx_dev = 2 * fx + px
    by_dev = 2 * fy + py
    counted = (fx >= 0) & (fx < 16) & (fy >= 0) & (fy < 16)
    is_real = np.zeros(NB * N_CORES, dtype=bool)
    for c in range(N_CORES):
        is_real[c * NB : c * NB + n_shard] = True
    gx_t = _true_bins(np.concatenate(
        [targ[c * n_shard : (c + 1) * n_shard, 0] for c in range(N_CORES)]))
    gy_t = _true_bins(np.concatenate(
        [targ[c * n_shard : (c + 1) * n_shard, 1] for c in range(N_CORES)]))
    bx_r = bx_dev[is_real]
    by_r = by_dev[is_real]
    cnt_r = counted[is_real]
    ok = cnt_r & (bx_r == gx_t) & (by_r == gy_t)
    sub_x = np.concatenate([bx_r[cnt_r & ~ok], bx_dev[~is_real & counted]])
    sub_y = np.concatenate([by_r[cnt_r & ~ok], by_dev[~is_real & counted]])
    np.subtract.at(hist, (sub_y, sub_x), 1.0)
    np.add.at(hist, (gy_t[~ok], gx_t[~ok]), 1.0)
    assert hist.sum() == N, (hist.sum(), N)

    mean_base = base_sum / N
    max_h = hist.max()
    result = mean_base * (1.0 + ALPHA * (N / (GRID * GRID)) / max_h)
    return np.float32(result)


# revision 23
# speedup vs baseline: 1.5395x; 1.1142x over previous
"""DOSAConLoss Trainium2 kernel (v3).

result = mean(base) * (1 + ALPHA * (N/1024) / max_hist)
since sum(hist) == N exactly (every box center lands in one bin).

8-way data parallel over N. Host ships inputs as fp16 PLANAR [4, NB] per
core (x/y/w/h planes) — halves transfer bytes and makes per-plane SBUF
slices dense.

Per core, per tile of 128x512 boxes:
  - CIoU chain in f32 (bf16 where precision allows), atan via
      atan(a)-atan(b) = atan((w2*h1-w1*h2)/(h1*h2+w1*w2))
    (valid since both atans are in (0, pi/2)); one Arctan per tile.
  - base = (1-ciou)^3 / (w2*h2+1e-7) accumulated into acc_out[128, nt].
  - Histogram, radix-64 packed: per box, z = 16*v; bin-half j = floor(z)
    (16 bins), parity p = [frac >= .5]; weight wx = 1+63*px (x side),
    wy = 1+4095*py (y side), so the PE product carries the 2x2 subcell
    in base-64 digits (1, 64, 4096, 262144; counts < 64 per psum group).
    A custom DVE op (ONEHOT16W, registered at import into
    concourse.dve_ops) builds the weighted one-hot directly in t-major
    layout: out[t*16+j] = [0 <= z[t]-j < 1] * win[t] — one instruction
    per side per tile. t-major makes 8 consecutive box-columns a
    contiguous [128,128] block, so the PE does 64 stacked matmuls per
    tile (instead of 4096 tiny LDW-bound ones), accumulating all
    (t mod 8)-diagonal blocks into one psum[128,128] per tile.
Host decodes base-64 digits from the 8 diagonal 16x16 blocks per tile,
replicates the device's fp16 binning exactly in numpy, and moves boxes
whose fp16 bin differs from the f32 reference bin (~1%), making the
final histogram exact. Reciprocals via exp(-ln(x)) on ACT.
"""

import numpy as np

import concourse.bass as bass
import concourse.bacc as bacc
import concourse.mybir as mybir
import concourse.tile as tile
from concourse import bass_utils
from concourse import dve_ops as _dve_ops
from concourse.dve_spec import (
    AluOp as _AluOp, Bin as _Bin, Idx as _Idx, PageIdx as _PageIdx,
    Spec as _Spec, Src0 as _Src0, Src1 as _Src1, Zero as _Zero, One as _One,
    C0 as _C0, lower as _dve_lower, _has_src1,
)
from concourse.dve_uop import DveOpSpec as _DveOpSpec

# Keep Ln+Exp in one act table (natural_log_exp_and_others): hide them
# from the single-function sets so the chooser lands on the joint one.
_orig_get_act_tables = bacc.get_activation_tables


def _patched_get_act_tables(arch):
    t = {k: set(v) for k, v in _orig_get_act_tables(arch).items()}
    t.get("natural_log", set()).discard(mybir.ActivationFunctionType.Ln)
    t.get("exp_and_others", set()).discard(mybir.ActivationFunctionType.Exp)
    t.get("exp_and_friends", set()).discard(mybir.ActivationFunctionType.Exp)
    return t


bacc.get_activation_tables = _patched_get_act_tables


# ---- custom DVE op: t-major weighted 16-bin one-hot ----------------------
# out[p, t*16+j] = [0 <= in0[p,t]-j < 1] * in1[p,t]   (j = Idx - 16*page)
def _onehot_ref(in0, in1, s0, s1, imm2):
    P = in0.shape[0]
    z = in0.astype(np.float32).reshape(P, -1)
    w = in1.astype(np.float32).reshape(P, -1)
    S, T = int(s0), z.shape[1]
    out = np.zeros((P, T * S), np.float32)
    for t in range(T):
        t0 = z[:, t, None] - np.arange(S, dtype=np.float32)[None, :]
        m = (t0 >= 0.0) & (t0 < 1.0)
        out[:, t * S:(t + 1) * S] = m * w[:, t, None]
    return out


def _register_onehot_op():
    if "ONEHOT16W" in _dve_ops._SUB_OPCODE_FOR_NAME:
        return [op for op in _dve_ops.OPS if op.name == "ONEHOT16W"][0]
    _j = _Bin(_AluOp.SUBTRACT, _Idx, _PageIdx(_Zero, _C0))
    _t0 = _Bin(_AluOp.SUBTRACT, _Src0, _j)
    _m = _Bin(_AluOp.MULTIPLY, _t0 >= _Zero, _t0 < _One)
    spec = _Spec(body=_Bin(_AluOp.MULTIPLY, _m, _Src1), reference=_onehot_ref)
    row = max(_dve_ops._SUB_OPCODE_FOR_NAME.values()) + 1
    assert row < 0x20
    op = _dve_ops.DveOp("ONEHOT16W", spec, subdim=True, uops_sha={})
    _dve_ops.OPS.append(op)
    _dve_ops._SUB_OPCODE_FOR_NAME[op.name] = row
    _dve_ops.CUSTOM_DVE_SPECS[op.name] = spec
    for ver in ("v3", "v4"):
        _dve_ops._COMPILE_CACHE[(op.name, ver)] = _DveOpSpec(
            name=op.name, opcode=row, uops=_dve_lower(spec, ver=ver),
            rd1_en=_has_src1(spec),
        )
    return op


ONEHOT16W = _register_onehot_op()

F32 = mybir.dt.float32
BF16 = mybir.dt.bfloat16
FP16 = mybir.dt.float16
AF = mybir.ActivationFunctionType
OP = mybir.AluOpType

GRID = 32
ALPHA = 1.5
EPS = 1e-7
PI = float(np.pi)
MAGIC = float(2 ** 23)

N_CORES = 8
N_TOTAL = 4_000_000
T = 512
TILE_BOX = 128 * T
NB_CORE = 524_288
# pred==targ -> base ~1e-21; x=y=1.0 -> z=16 -> one-hot match fails, so
# pads never enter the device histogram (no radix-capacity risk)
PAD_BOX = (1.0, 1.0, 1.0, 1.0)

# ops routed to the (slow but otherwise idle) GPSIMD Pool engine
GPS_OPS = {"asum", "cw2", "ch2", "scw", "sch", "c24", "iou", "iw4", "ih4", "den0"}


def build_nc(NB, T=T, Tc=None):
    n_tiles = NB // (128 * T)
    assert NB == n_tiles * 128 * T and T % 8 == 0

    nc = bacc.Bacc("TRN2", target_bir_lowering=False, debug=False)
    pred_d = nc.dram_tensor("pred_boxes", [4, NB], FP16, kind="ExternalInput")
    targ_d = nc.dram_tensor("target_boxes", [4, NB], FP16, kind="ExternalInput")
    acc_d = nc.dram_tensor("acc_out", [128, n_tiles], F32, kind="ExternalOutput")
    hist_d = nc.dram_tensor("hist_out", [128, n_tiles * 128], F32, kind="ExternalOutput")

    pred_v = pred_d.ap().rearrange("c (n p t) -> n p c t", p=128, t=T)
    targ_v = targ_d.ap().rearrange("c (n p t) -> n p c t", p=128, t=T)

    def eng(name):
        return nc.gpsimd if name in GPS_OPS else nc.vector

    with tile.TileContext(nc) as tc:
        with (
            tc.tile_pool(name="inp", bufs=2) as inp,
            tc.tile_pool(name="tmp", bufs=2) as tmp,
            tc.tile_pool(name="ohp", bufs=2) as ohp,
            tc.tile_pool(name="cst", bufs=1) as cst,
            tc.tile_pool(name="psp", bufs=1, space="PSUM") as psp,
        ):
            bias_tiles = {}

            def bias_ap(val):
                if val not in bias_tiles:
                    t_ = cst.tile([128, 1], F32, name=f"bias{len(bias_tiles)}")
                    nc.vector.memset(t_[:], val)
                    bias_tiles[val] = t_[:]
                return bias_tiles[val]

            acc_sb = cst.tile([128, n_tiles], F32)
            hist_sb = cst.tile([128, n_tiles * 128], F32)
            ps = [psp.tile([128, 128], F32, name=f"ps{g}") for g in range(n_tiles)]

            NGEN = 14
            DEDICATED = {"a2t", "iou", "term1", "vv", "z"}
            gen_counter = [0]

            for n in range(n_tiles):
                pt = inp.tile([128, 4 * T], FP16, tag="pred")
                tt = inp.tile([128, 4 * T], FP16, tag="targ")
                p3 = pt.rearrange("p (c t) -> p c t", c=4)
                t3 = tt.rearrange("p (c t) -> p c t", c=4)
                nc.sync.dma_start(p3[:, :, :], pred_v[n])
                nc.sync.dma_start(t3[:, :, :], targ_v[n])
                x1, y1, w1, h1 = p3[:, 0], p3[:, 1], p3[:, 2], p3[:, 3]
                x2, y2, w2, h2 = t3[:, 0], t3[:, 1], t3[:, 2], t3[:, 3]

                def t_(tag, dt=F32):
                    if tag in DEDICATED:
                        return tmp.tile([128, T], dt, tag=tag, name=tag)[:]
                    i = gen_counter[0] % NGEN
                    gen_counter[0] += 1
                    return tmp.tile([128, T], dt, tag=f"g{i}", name=tag)[:]

                # ---- histogram prep + one-hots (early: feeds PE) ----
                # x (moving, m-major): fx = trickfloor(16x) via magic-RNE,
                #   parity px = [16x+0.5-fx >= 1]; pure one-hot [fx==j].
                # y (stationary, t-major custom op): zy = 16y; match
                #   j = floor(zy); parity py = [zy - RNE(zy) < 0]; the op
                #   carries the combined weight wc=(1+63px)(1+4095py).
                zx5 = t_("zx5")
                fxb = tmp.tile([128, T], BF16, tag="fxb", name="fxb")[:]
                pxb = tmp.tile([128, T], BF16, tag="pxb", name="pxb")[:]
                pyb = tmp.tile([128, T], BF16, tag="pyb", name="pyb")[:]
                txw = tmp.tile([128, T], BF16, tag="txw", name="txw")[:]
                tyw = tmp.tile([128, T], BF16, tag="tyw", name="tyw")[:]
                wcb = tmp.tile([128, T], BF16, tag="wcb", name="wcb")[:]
                frx = t_("frx")
                nc.scalar.activation(zx5, x2, AF.Identity, scale=16.0,
                                     bias=bias_ap(0.5))
                nc.vector.tensor_scalar(fxb, zx5, MAGIC, MAGIC + 1.0, OP.add, OP.subtract)
                nc.vector.scalar_tensor_tensor(frx, fxb, -1.0, zx5, OP.mult, OP.add)
                nc.vector.tensor_scalar(pxb, frx, 1.0, None, OP.is_ge)
                zy = tmp.tile([128, T], F32, tag="zy", name="zy")
                fly, dyt = t_("fly"), t_("dyt")
                nc.scalar.activation(zy[:], y2, AF.Identity, scale=16.0)
                nc.vector.tensor_scalar(fly, zy[:], MAGIC, MAGIC, OP.add, OP.subtract)
                nc.vector.scalar_tensor_tensor(dyt, fly, -1.0, zy[:], OP.mult, OP.add)
                nc.vector.tensor_scalar(pyb, dyt, 0.0, None, OP.is_lt)
                nc.vector.tensor_scalar(txw, pxb, 63.0, 1.0, OP.mult, OP.add)
                nc.vector.tensor_scalar(tyw, pyb, 4095.0, 1.0, OP.mult, OP.add)
                nc.vector.tensor_tensor(wcb, txw, tyw, OP.mult)

                ohx = ohp.tile([128, 16 * T], BF16, tag="ohx", name="ohx")
                ohy = ohp.tile([128, T * 16], BF16, tag="ohy", name="ohy")
                for j in range(16):
                    nc.vector.tensor_scalar(
                        ohx[:, j * T : (j + 1) * T], fxb, float(j), None, OP.is_equal
                    )
                oy3 = ohy.rearrange("p (t j) -> p t j", j=16)
                nc.vector._custom_dve(
                    ONEHOT16W, out=oy3[:, :, :],
                    in0=zy[:].unsqueeze(2).broadcast_to([128, T, 16]),
                    in1=wcb.unsqueeze(2).broadcast_to([128, T, 16]),
                    s0=16.0,
                )
                # moving operand dims (j, ti) with ti innermost (unit
                # stride): 8-element bursts keep the PE stream fed.
                # psum column q = 8*j + ti.
                ox3 = ohx.rearrange("p (j t) -> p j t", t=T)
                n_mm = T // 8
                for k in range(n_mm):
                    nc.tensor.matmul(
                        ps[n][:],
                        ohy[:, 128 * k : 128 * k + 128],
                        ox3[:, :, 8 * k : 8 * k + 8],
                        start=(k == 0), stop=(k == n_mm - 1),
                    )
                nc.vector.tensor_copy(hist_sb[:, n * 128 : (n + 1) * 128], ps[n][:])

                # ---- CIoU chain ----
                dx, dy = t_("dx"), t_("dy")
                W, dW, H, dH = t_("W"), t_("dW"), t_("H"), t_("dH")
                nc.vector.tensor_tensor(dx, x1, x2, OP.subtract)
                nc.vector.tensor_tensor(dy, y1, y2, OP.subtract)
                nc.vector.tensor_tensor(W, w1, w2, OP.add)
                nc.vector.tensor_tensor(dW, w1, w2, OP.subtract)
                nc.vector.tensor_tensor(H, h1, h2, OP.add)
                nc.vector.tensor_tensor(dH, h1, h2, OP.subtract)
                a2t, a1t, asum = t_("a2t"), t_("a1t"), t_("asum")
                nc.vector.tensor_tensor(a2t, w2, h2, OP.mult)
                nc.vector.tensor_tensor(a1t, w1, h1, OP.mult)
                eng("asum").tensor_tensor(asum, a1t, a2t, OP.add)

                adx, ady, adW, adH = t_("adx"), t_("ady"), t_("adW"), t_("adH")
                nc.scalar.activation(adx, dx, AF.Abs, scale=2.0)
                nc.scalar.activation(ady, dy, AF.Abs, scale=2.0)
                nc.scalar.activation(adW, dW, AF.Abs)
                nc.scalar.activation(adH, dH, AF.Abs)
                mx, my = t_("mx"), t_("my")
                nc.vector.tensor_tensor(mx, adx, adW, OP.max)
                nc.vector.tensor_tensor(my, ady, adH, OP.max)

                iw4, ih4, ihc, inter4 = t_("iw4"), t_("ih4"), t_("ihc"), t_("inter4")
                eng("iw4").tensor_tensor(iw4, W, mx, OP.subtract)
                eng("ih4").tensor_tensor(ih4, H, my, OP.subtract)
                nc.scalar.activation(ihc, ih4, AF.Relu)
                nc.vector.scalar_tensor_tensor(inter4, iw4, 0.0, ihc, OP.max, OP.mult)
                u4 = t_("u4")
                nc.vector.scalar_tensor_tensor(u4, inter4, -0.25, asum, OP.mult, OP.add)
                lnu, r_u = t_("lnu"), t_("r_u")
                nc.scalar.activation(lnu, u4, AF.Ln, scale=4.0, bias=bias_ap(4 * EPS))
                nc.scalar.activation(r_u, lnu, AF.Exp, scale=-1.0)
                iou = t_("iou")
                eng("iou").tensor_tensor(iou, inter4, r_u, OP.mult)

                def tb(tag):
                    return tmp.tile([128, T], BF16, tag=f"b_{tag}", name=tag)[:]

                cw2, ch2 = t_("cw2"), t_("ch2")
                eng("cw2").tensor_tensor(cw2, W, mx, OP.add)
                eng("ch2").tensor_tensor(ch2, H, my, OP.add)
                scw, sch = t_("scw"), t_("sch")
                sdx, sdy = tb("sdx"), tb("sdy")
                eng("scw").tensor_tensor(scw, cw2, cw2, OP.mult)
                eng("sch").tensor_tensor(sch, ch2, ch2, OP.mult)
                nc.scalar.activation(sdx, dx, AF.Square, scale=2.0)
                nc.scalar.activation(sdy, dy, AF.Square, scale=2.0)
                c24 = t_("c24")
                rho4 = tb("rho4")
                eng("c24").tensor_tensor(c24, scw, sch, OP.add)
                nc.vector.tensor_tensor(rho4, sdx, sdy, OP.add)
                lnc = t_("lnc")
                r_c = tb("r_c")
                nc.scalar.activation(lnc, c24, AF.Ln, bias=bias_ap(4 * EPS))
                nc.scalar.activation(r_c, lnc, AF.Exp, scale=-1.0)
                term1 = tb("term1")
                nc.vector.tensor_tensor(term1, rho4, r_c, OP.mult)

                # atan(w2/h2)-atan(w1/h1) = atan((w2*h1-w1*h2)/(h1*h2+w1*w2))
                c1, c2, d1, d2 = tb("c1"), tb("c2"), tb("d1"), tb("d2")
                nc.vector.tensor_tensor(c1, w2, h1, OP.mult)
                nc.vector.tensor_tensor(c2, w1, h2, OP.mult)
                nc.vector.tensor_tensor(d1, h1, h2, OP.mult)
                nc.vector.tensor_tensor(d2, w1, w2, OP.mult)
                numq, den = tb("numq"), tb("den")
                nc.vector.tensor_tensor(numq, c1, c2, OP.subtract)
                nc.vector.tensor_tensor(den, d1, d2, OP.add)
                lnd = t_("lnd")
                r_d = tb("r_d")
                nc.scalar.activation(lnd, den, AF.Ln, bias=bias_ap(1e-30))
                nc.scalar.activation(r_d, lnd, AF.Exp, scale=-1.0)
                lnsw, sw = t_("lnsw"), t_("sw")
                nc.scalar.activation(lnsw, a2t, AF.Ln, bias=bias_ap(1e-7))
                nc.scalar.activation(sw, lnsw, AF.Exp, scale=-1.0)
                q = tb("q")
                at = t_("at")
                nc.vector.tensor_tensor(q, numq, r_d, OP.mult)
                nc.scalar.activation(at, q, AF.Arctan)
                vv = t_("vv")
                nc.scalar.activation(vv, at, AF.Square, scale=2.0 / PI)

                den0 = t_("den0")
                eng("den0").tensor_tensor(den0, vv, iou, OP.subtract)
                lnden = t_("lnden")
                rden, v2 = tb("rden"), tb("v2")
                nc.scalar.activation(lnden, den0, AF.Ln, bias=bias_ap(1.0 + EPS))
                nc.scalar.activation(rden, lnden, AF.Exp, scale=-1.0)
                nc.scalar.activation(v2, vv, AF.Square)
                term2, s12 = tb("term2"), tb("s12")
                z = t_("z")
                nc.vector.tensor_tensor(term2, v2, rden, OP.mult)
                nc.vector.tensor_tensor(s12, term1, term2, OP.add)
                nc.vector.scalar_tensor_tensor(z, iou, -1.0, s12, OP.mult, OP.add)

                om2, om3 = t_("om2"), t_("om3")
                nc.scalar.activation(om2, z, AF.Square, bias=bias_ap(1.0))
                nc.vector.scalar_tensor_tensor(om3, z, 1.0, om2, OP.add, OP.mult)
                nc.vector.scalar_tensor_tensor(
                    t_("baset"), om3, 0.0, sw, OP.add, OP.mult,
                    accum_out=acc_sb[:, n : n + 1],
                )

            nc.sync.dma_start(hist_d.ap(), hist_sb[:])
            nc.sync.dma_start(acc_d.ap(), acc_sb[:])

    nc.compile()
    return nc


_CACHE = {}
RUN_KW = {}
LAST_RESULT = None


def _get_program(NB, T_=T, Tc=None):
    key = (NB, T_)
    if key not in _CACHE:
        _CACHE[key] = build_nc(NB, T=T_)
    return _CACHE[key]


def _dev_bins_x(v16):
    """x-side device binning: z5 = 16v+0.5; fx = RNE(z5+M)-(M+1);
    px = [z5 - fx >= 1]. Counted iff 0 <= fx < 16."""
    z5 = (np.float32(16.0) * v16.astype(np.float32) + np.float32(0.5)).astype(np.float32)
    fx = ((z5 + np.float32(MAGIC)).astype(np.float32)
          - np.float32(MAGIC + 1.0)).astype(np.float32)
    px = ((z5 - fx) >= np.float32(1.0)).astype(np.int64)
    return fx.astype(np.int64), px


def _dev_bins_y(v16):
    """y-side device binning (custom op): z = 16v; j = floor(z) matched
    iff 0<=j<16; parity p = [z - RNE(z) < 0]."""
    z = np.float32(16.0) * v16.astype(np.float32)
    fl = ((z + np.float32(MAGIC)) - np.float32(MAGIC)).astype(np.float32)
    p = ((z - fl) < 0).astype(np.int64)
    return np.floor(z).astype(np.int64), p


def _true_bins(v32):
    return np.clip((v32 * np.float32(GRID)).astype(np.int32), 0, GRID - 1).astype(np.int64)


def _decode_hists(raw_list, n_tiles):
    """Per-core [128, n_tiles*128] psum dumps -> exact device histogram.
    Diagonal 16x16 blocks hold base-64 digit-packed counts."""
    hist = np.zeros((GRID, GRID), dtype=np.float64)
    for raw in raw_list:
        R = raw.reshape(128, n_tiles, 128).astype(np.float64)
        for g in range(n_tiles):
            P = R[:, g, :]
            for ti in range(8):
                # stationary rows p = 16*ti + m; moving cols q = 8*j + ti
                D = P[16 * ti : 16 * ti + 16, ti::8]
                d3 = np.floor(D / 262144.0)
                r = D - d3 * 262144.0
                d2 = np.floor(r / 4096.0)
                r -= d2 * 4096.0
                d1 = np.floor(r / 64.0)
                d0 = r - d1 * 64.0
                for dd in (d0, d1, d2, d3):
                    assert (dd >= 0).all() and (dd < 64).all(), "radix overflow"
                hist[0::2, 0::2] += d0
                hist[0::2, 1::2] += d1
                hist[1::2, 0::2] += d2
                hist[1::2, 1::2] += d3
    return hist


def kernel(pred_boxes: np.ndarray, target_boxes: np.ndarray) -> np.ndarray:
    N = pred_boxes.shape[0]
    assert N % N_CORES == 0
    n_shard = N // N_CORES
    NB = ((n_shard + TILE_BOX - 1) // TILE_BOX) * TILE_BOX
    n_tiles = NB // TILE_BOX
    pad = NB - n_shard

    pred = np.asarray(pred_boxes, dtype=np.float32)
    targ = np.asarray(target_boxes, dtype=np.float32)

    padrow = np.array(PAD_BOX, dtype=np.float16)
    in_maps = []
    targ16 = []
    for c in range(N_CORES):
        pm = np.empty((4, NB), dtype=np.float16)
        tm = np.empty((4, NB), dtype=np.float16)
        pm[:, :n_shard] = pred[c * n_shard : (c + 1) * n_shard].T
        tm[:, :n_shard] = targ[c * n_shard : (c + 1) * n_shard].T
        if pad:
            pm[:, n_shard:] = padrow[:, None]
            tm[:, n_shard:] = padrow[:, None]
        in_maps.append({"pred_boxes": pm, "target_boxes": tm})
        targ16.append(tm)

    nc = _get_program(NB)
    res = bass_utils.run_bass_kernel_spmd(
        nc, in_maps, core_ids=list(range(N_CORES)), **RUN_KW
    )
    global LAST_RESULT
    LAST_RESULT = res

    base_sum = 0.0
    raws = []
    for r in res.results:
        base_sum += float(r["acc_out"].astype(np.float64).sum())
        raws.append(r["hist_out"])
    hist = _decode_hists(raws, n_tiles)

    # exact fixup: move boxes whose device (fp16) bin differs from the
    # f32 reference bin; drop pads; add device-dropped boxes (j>15)
    x16 = np.concatenate([t[0] for t in targ16])
    y16 = np.concatenate([t[1] for t in targ16])
    fx, px = _dev_bins_x(x16)
    fy, py = _dev_bins_y(y16)
    bx_dev = 2 * fx + px
    by_dev = 2 * fy + py
    counted = (fx >= 0) & (fx < 16) & (fy >= 0) & (fy < 16)
    is_real = np.zeros(NB * N_CORES, dtype=bool)
    for c in range(N_CORES):
        is_real[c * NB : c * NB + n_shard] = True
    gx_t = _true_bins(np.concatenate(
        [targ[c * n_shard : (c + 1) * n_shard, 0] for c in range(N_CORES)]))
    gy_t = _true_bins(np.concatenate(
        [targ[c * n_shard : (c + 1) * n_shard, 1] for c in range(N_CORES)]))
    bx_r = bx_dev[is_real]
    by_r = by_dev[is_real]
    cnt_r = counted[is_real]
    ok = cnt_r & (bx_r == gx_t) & (by_r == gy_t)
    sub_x = np.concatenate([bx_r[cnt_r & ~ok], bx_dev[~is_real & counted]])
    sub_y = np.concatenate([by_r[cnt_r & ~ok], by_dev[~is_real & counted]])
    np.subtract.at(hist, (sub_y, sub_x), 1.0)
    np.add.at(hist, (gy_t[~ok], gx_t[~ok]), 1.0)
    assert hist.sum() == N, (hist.sum(), N)

    mean_base = base_sum / N
    max_h = hist.max()
    result = mean_base * (1.0 + ALPHA * (N / (GRID * GRID)) / max_h)
    return np.float32(result)
